# revision 2
# baseline (speedup 1.0000x reference)
"""GraphSAGE 2-layer on 8 trn2 cores — v4: window-bucketed per-edge A2A halo.

Like v3 (per-edge AllToAll cells, one-hot scatter, no receiver gathers), but
every cell is subdivided into NWIN window-buckets of SHARED size BKT[w]
(max over all (core, shard) pairs, 128-aligned).  The receiver then works
window-major: for window w it streams the 8 cells' w-buckets (plain DMAs),
scatter-matmuls every 128-row tile into ONE accumulating PSUM window, and
flushes with a single tensor_copy into an fp16 agg.  Piece count == tile
count (no cross-core union), no flush adds, no agg memset.
Pad rows are masked by all-zero one-hot columns (dcmp == -1).
"""
import sys
sys.path.insert(0, '/opt/trn_rl_repo')

import numpy as np

N = 200000
E = 600000
C = 8
P = N // C
IN_DIM = 130
HID = 128
BN_EPS = 1e-5
WIN = 512
SL = 25088
NWIN = SL // WIN           # 49
F16 = np.float16

_cache = {}


def _round_up(x, m):
    return (x + m - 1) // m * m


def _wrap16(flat):
    s = len(flat)
    blk = np.asarray(flat, np.int16).reshape(s // 16, 16).T
    return np.tile(blk, (8, 1))


def _preprocess_graph(edge_index):
    src = edge_index[0].astype(np.int64)
    dst = edge_index[1].astype(np.int64)
    owner = dst // P
    dloc = dst - owner * P
    jsh = src // P
    wnd = dloc // WIN
    deg = np.bincount(dst, minlength=N)
    invd = (1.0 / np.maximum(deg, 1)).astype(np.float32)

    # bucket sizes shared across every (core, shard) pair
    cnt = np.zeros((C, C, NWIN), np.int64)
    np.add.at(cnt, (owner, jsh, wnd), 1)
    BKT = ((cnt.max(axis=(0, 1)) + 127) // 128) * 128      # [NWIN]
    OFF = np.concatenate(([0], np.cumsum(BKT)))            # [NWIN+1]
    CELLJ = int(OFF[-1])
    ACELL = C * CELLJ

    # order edges by (owner, shard, window, dloc); rank within run
    eorder = np.lexsort((dloc, wnd, jsh, owner))
    o_s, j_s, w_s, d_s, s_s = (owner[eorder], jsh[eorder], wnd[eorder],
                               dloc[eorder], src[eorder])
    key = (o_s * C + j_s) * NWIN + w_s
    bounds = np.searchsorted(key, np.arange(C * C * NWIN + 1))

    # per-halo-row tables (receiver view, core c)
    dcmp = np.full((C, ACELL), -1.0, np.float32)
    dinv = np.zeros((C, ACELL), np.float32)
    # sender view (core j): a2a_in row -> global src / src slot
    sendsrc = np.full((C, ACELL), -1, np.int64)
    sendflat = np.full((C, ACELL), SL, np.int64)
    for c in range(C):
        for j in range(C):
            for w in range(NWIN):
                k = (c * C + j) * NWIN + w
                a, b = bounds[k], bounds[k + 1]
                n = b - a
                base = j * CELLJ + OFF[w]
                dcmp[c, base:base + n] = d_s[a:b] - w * WIN
                dinv[c, base:base + n] = invd[c * P + d_s[a:b]]
                sbase = c * CELLJ + OFF[w]
                sendsrc[j, sbase:sbase + n] = s_s[a:b]
                sendflat[j, sbase:sbase + n] = s_s[a:b] - j * P
    sendidx = np.stack([_wrap16(sendflat[j]) for j in range(C)])

    # pieces: window-major, (w, j, t); NP = sum BKT/128 * C
    NP = int(BKT.sum() // 128) * C
    dcmp_in = np.empty((C, 128, NP), np.float32)
    dinv_in = np.empty((C, 128, NP), np.float32)
    pc = 0
    piece_plan = []            # (w, j, t, pc)
    for w in range(NWIN):
        for j in range(C):
            for t in range(int(BKT[w]) // 128):
                base = j * CELLJ + OFF[w] + t * 128
                dcmp_in[:, :, pc] = dcmp[:, base:base + 128]
                dinv_in[:, :, pc] = dinv[:, base:base + 128]
                piece_plan.append((w, j, t, pc))
                pc += 1
    assert pc == NP

    return dict(CELLJ=CELLJ, ACELL=ACELL, NP=NP, BKT=BKT.astype(int),
                OFF=OFF.astype(int), piece_plan=piece_plan,
                dcmp_in=dcmp_in, dinv_in=dinv_in,
                sendsrc=sendsrc, sendflat=sendflat, sendidx=sendidx)


def _fold_weights(Wl, bl, Wr, g, be, rm, rv):
    s = (np.asarray(g) / np.sqrt(np.asarray(rv) + BN_EPS)).astype(np.float32)
    Wl_f = (np.asarray(Wl) * s[None, :]).astype(np.float32)
    Wr_f = (np.asarray(Wr) * s[None, :]).astype(np.float32)
    c = ((np.asarray(bl) - np.asarray(rm)) * s + np.asarray(be)).astype(np.float32)
    return Wl_f, Wr_f, c


def _make_in_maps(inputs, g):
    x = np.asarray(inputs['x'], np.float32)
    W1l, W1r, c1 = _fold_weights(inputs['W1_l'], inputs['b1_l'], inputs['W1_r'],
                                 inputs['g1'], inputs['be1'], inputs['rm1'],
                                 inputs['rv1'])
    W2l, W2r, c2 = _fold_weights(inputs['W2_l'], inputs['b2_l'], inputs['W2_r'],
                                 inputs['g2'], inputs['be2'], inputs['rm2'],
                                 inputs['rv2'])
    ACELL = g['ACELL']
    shared = {
        'W1la': W1l[0:128].astype(F16), 'W1lb': W1l[128:130].astype(F16),
        'W1ra': W1r[0:128].astype(F16), 'W1rb': W1r[128:130].astype(F16),
        'W2l': W2l.astype(F16), 'W2r': W2r.astype(F16),
        'c1': c1.reshape(128, 1), 'c2': c2.reshape(128, 1),
    }
    in_maps = []
    for c in range(C):
        m = dict(shared)
        xn = np.zeros((IN_DIM, SL), np.float32)
        xn[:, :P] = x[c * P:(c + 1) * P].T
        m['xna'] = xn[0:128].astype(F16)
        m['xnb'] = xn[128:130].astype(F16)
        xc = np.zeros((IN_DIM, ACELL), np.float32)
        sel = g['sendsrc'][c] >= 0
        xc[:, sel] = x[g['sendsrc'][c][sel]].T
        m['xca'] = xc[0:128].astype(F16)
        m['xcb'] = xc[128:130].astype(F16)
        m['dcmp'] = g['dcmp_in'][c]
        m['dinv'] = g['dinv_in'][c]
        m['iota'] = np.tile(np.arange(WIN, dtype=np.float32), (128, 1))
        m['sendidx'] = g['sendidx'][c]
        in_maps.append(m)
    return in_maps


def _assemble_output(outs, g):
    h2 = np.empty((N, HID), np.float32)
    for c in range(C):
        h2[c * P:(c + 1) * P] = outs[c]['hout'][:, :P].T.astype(np.float32)
    return h2


# ---------------------------------------------------------------------------
# numpy emulation
# ---------------------------------------------------------------------------

def _emulate_full(in_maps, g):
    CELLJ = g['CELLJ']

    def f16(a):
        return a.astype(F16).astype(np.float32)

    W1la = in_maps[0]['W1la'].astype(np.float32)
    W1lb = in_maps[0]['W1lb'].astype(np.float32)
    W1ra = in_maps[0]['W1ra'].astype(np.float32)
    W1rb = in_maps[0]['W1rb'].astype(np.float32)
    W2l = in_maps[0]['W2l'].astype(np.float32)
    W2r = in_maps[0]['W2r'].astype(np.float32)
    c1 = in_maps[0]['c1'].ravel()
    c2 = in_maps[0]['c2'].ravel()

    def aggregate(zin_rows):
        halo = [np.concatenate([zin_rows[j][c * CELLJ:(c + 1) * CELLJ]
                                for j in range(C)]) for c in range(C)]
        aggs = []
        iota = np.arange(WIN, dtype=np.float32)
        for c in range(C):
            agg = np.zeros((128, SL), np.float32)
            dc = in_maps[c]['dcmp'].astype(np.float32)
            di = in_maps[c]['dinv'].astype(np.float32)
            for (w, j, t, pc) in g['piece_plan']:
                base = j * CELLJ + g['OFF'][w] + t * 128
                tile = halo[c][base:base + 128]
                oh = f16((iota[None, :] == dc[:, pc:pc + 1]) * di[:, pc:pc + 1])
                agg[:, w * WIN:(w + 1) * WIN] += tile.T @ oh
            aggs.append(f16(agg))
        return aggs

    zcells = []
    for c in range(C):
        xca = in_maps[c]['xca'].astype(np.float32)
        xcb = in_maps[c]['xcb'].astype(np.float32)
        zcells.append(f16(xca.T @ W1la + xcb.T @ W1lb))
    aggs = aggregate(zcells)
    h1s = []
    z2locs = []
    for c in range(C):
        xna = in_maps[c]['xna'].astype(np.float32)
        xnb = in_maps[c]['xnb'].astype(np.float32)
        r1 = W1ra.T @ xna + W1rb.T @ xnb
        h1 = f16(np.maximum(aggs[c] + r1 + c1[:, None], 0.0))
        h1s.append(h1)
        z2 = np.concatenate([f16(h1.T @ W2l), np.zeros((16, 128), np.float32)])
        z2locs.append(z2)
    zcells2 = [z2locs[c][g['sendflat'][c]] for c in range(C)]
    aggs2 = aggregate(zcells2)
    outs = []
    for c in range(C):
        r2 = W2r.T @ h1s[c]
        h2 = f16(np.maximum(aggs2[c] + r2 + c2[:, None], 0.0))
        outs.append({'hout': h2.astype(F16)})
    return outs


# ---------------------------------------------------------------------------
# bass kernel
# ---------------------------------------------------------------------------

def _build_kernel(g, nreps=1):
    import concourse.bacc as bacc
    import concourse.tile as tile
    import concourse.mybir as mybir

    F32 = mybir.dt.float32
    FP16 = mybir.dt.float16
    I16 = mybir.dt.int16
    AF = mybir.ActivationFunctionType
    ALU = mybir.AluOpType

    ACELL, CELLJ, NP = g['ACELL'], g['CELLJ'], g['NP']
    BKT, OFF = g['BKT'], g['OFF']
    NKC = ACELL // 512
    NW = SL // 512
    maxbt = int(max(BKT)) // 128

    nc = bacc.Bacc("TRN2", target_bir_lowering=False, debug=False, num_devices=C,
                   num_swdge_queues=4)

    xna_d = nc.dram_tensor("xna", [128, SL], FP16, kind="ExternalInput")
    xnb_d = nc.dram_tensor("xnb", [2, SL], FP16, kind="ExternalInput")
    xca_d = nc.dram_tensor("xca", [128, ACELL], FP16, kind="ExternalInput")
    xcb_d = nc.dram_tensor("xcb", [2, ACELL], FP16, kind="ExternalInput")
    W1la_d = nc.dram_tensor("W1la", [128, 128], FP16, kind="ExternalInput")
    W1lb_d = nc.dram_tensor("W1lb", [2, 128], FP16, kind="ExternalInput")
    W1ra_d = nc.dram_tensor("W1ra", [128, 128], FP16, kind="ExternalInput")
    W1rb_d = nc.dram_tensor("W1rb", [2, 128], FP16, kind="ExternalInput")
    W2l_d = nc.dram_tensor("W2l", [128, 128], FP16, kind="ExternalInput")
    W2r_d = nc.dram_tensor("W2r", [128, 128], FP16, kind="ExternalInput")
    c1_d = nc.dram_tensor("c1", [128, 1], F32, kind="ExternalInput")
    c2_d = nc.dram_tensor("c2", [128, 1], F32, kind="ExternalInput")
    dcmp_d = nc.dram_tensor("dcmp", [128, NP], F32, kind="ExternalInput")
    dinv_d = nc.dram_tensor("dinv", [128, NP], F32, kind="ExternalInput")
    iota_d = nc.dram_tensor("iota", [128, WIN], F32, kind="ExternalInput")
    sidx_d = nc.dram_tensor("sendidx", [128, ACELL // 16], I16, kind="ExternalInput")
    hout_d = nc.dram_tensor("hout", [128, SL], FP16, kind="ExternalOutput")

    a2a_in = nc.dram_tensor("a2ain", [ACELL, 128], FP16)
    halo = nc.dram_tensor("halo", [ACELL, 128], FP16)
    z2loc = nc.dram_tensor("z2loc", [SL + 16, 128], FP16)
    h1loc = nc.dram_tensor("h1loc", [128, SL], FP16)

    with tile.TileContext(nc) as tc:
        with (
            tc.tile_pool(name="const", bufs=1) as cons,
            tc.tile_pool(name="big", bufs=1) as bigp,
            tc.tile_pool(name="xs", bufs=3) as xsp,
            tc.tile_pool(name="zt", bufs=3) as ztp,
            tc.tile_pool(name="hc", bufs=4) as hcp,
            tc.tile_pool(name="oh", bufs=4) as ohp,
            tc.tile_pool(name="ph", bufs=3) as php,
            tc.tile_pool(name="agps", bufs=3, space="PSUM") as agps,
            tc.tile_pool(name="ps1", bufs=3, space="PSUM") as ps1p,
        ):
            w1la = cons.tile([128, 128], FP16)
            w1lb = cons.tile([2, 128], FP16)
            w1ra = cons.tile([128, 128], FP16)
            w1rb = cons.tile([2, 128], FP16)
            w2l = cons.tile([128, 128], FP16)
            w2r = cons.tile([128, 128], FP16)
            c1t = cons.tile([128, 1], F32)
            c2t = cons.tile([128, 1], F32)
            dcmp_t = cons.tile([128, NP], F32)
            dinv_t = cons.tile([128, NP], F32)
            iota_t = cons.tile([128, WIN], F32)
            sidx = cons.tile([128, ACELL // 16], I16)
            for sb, dr in ((w1la, W1la_d), (w1lb, W1lb_d), (w1ra, W1ra_d),
                           (w1rb, W1rb_d), (w2l, W2l_d), (w2r, W2r_d),
                           (c1t, c1_d), (c2t, c2_d), (dcmp_t, dcmp_d),
                           (dinv_t, dinv_d), (iota_t, iota_d), (sidx, sidx_d)):
                nc.sync.dma_start(sb[:], dr[:])
            zrow = cons.tile([1, 128], FP16)
            nc.vector.memset(zrow[:], 0.0)
            nc.sync.dma_start(z2loc[SL:SL + 1, :], zrow[:])

            agg = bigp.tile([128, SL], FP16)

            def aggregate():
                pc = 0
                for w in range(NWIN):
                    ps = agps.tile([128, WIN], F32, tag="agps")
                    first = True
                    for j in range(C):
                        nt = int(BKT[w]) // 128
                        hch = hcp.tile([128, maxbt, 128], FP16, tag="hch")
                        base = j * CELLJ + int(OFF[w])
                        nc.sync.dma_start(
                            hch[:, 0:nt, :],
                            halo[base:base + nt * 128, :].rearrange(
                                "(b p) f -> p b f", p=128))
                        for t in range(nt):
                            oh = ohp.tile([128, WIN], FP16, tag="oh")
                            nc.vector.tensor_scalar(
                                oh[:], iota_t[:],
                                dcmp_t[:, pc:pc + 1], dinv_t[:, pc:pc + 1],
                                op0=ALU.is_equal, op1=ALU.mult)
                            nc.tensor.matmul(ps[:], lhsT=hch[:, t, :], rhs=oh[:],
                                             start=first, stop=False,
                                             skip_group_check=True)
                            first = False
                            pc += 1
                    nc.vector.tensor_copy(agg[:, w * WIN:(w + 1) * WIN], ps[:])
                assert pc == NP

            for rep in range(nreps):
                # ---- phase A: z1 per edge-slot -> a2a_in ----
                for k in range(NKC):
                    xa = xsp.tile([128, 512], FP16, tag="xa")
                    xb = xsp.tile([2, 512], FP16, tag="xb")
                    nc.sync.dma_start(xa[:], xca_d[:, k * 512:(k + 1) * 512])
                    nc.sync.dma_start(xb[:], xcb_d[:, k * 512:(k + 1) * 512])
                    zt = ztp.tile([128, 4, 128], FP16, tag="zt")
                    for b in range(4):
                        ps = ps1p.tile([128, 512], F32, tag="ps1")
                        nc.tensor.matmul(ps[:, 0:128], lhsT=xa[:, b * 128:(b + 1) * 128],
                                         rhs=w1la[:], start=True, stop=False)
                        nc.tensor.matmul(ps[:, 0:128], lhsT=xb[:, b * 128:(b + 1) * 128],
                                         rhs=w1lb[:], start=False, stop=True)
                        nc.scalar.copy(zt[:, b, :], ps[:, 0:128])
                    nc.sync.dma_start(
                        a2a_in[k * 512:(k + 1) * 512, :].rearrange(
                            "(b p) f -> p b f", p=128), zt[:])

                nc.gpsimd.collective_compute(
                    "AllToAll", mybir.AluOpType.bypass,
                    replica_groups=[list(range(C))],
                    ins=[a2a_in[:].opt()], outs=[halo[:].opt()])

                aggregate()

                # ---- phase D ----
                for w in range(NW):
                    sl = slice(w * 512, (w + 1) * 512)
                    xa = xsp.tile([128, 512], FP16, tag="xa")
                    xb = xsp.tile([2, 512], FP16, tag="xb")
                    nc.sync.dma_start(xa[:], xna_d[:, sl])
                    nc.sync.dma_start(xb[:], xnb_d[:, sl])
                    ps2 = ps1p.tile([128, 512], F32, tag="ps1")
                    nc.tensor.matmul(ps2[:], lhsT=w1ra[:], rhs=xa[:],
                                     start=True, stop=False)
                    nc.tensor.matmul(ps2[:], lhsT=w1rb[:], rhs=xb[:],
                                     start=False, stop=True)
                    u2 = php.tile([128, 512], F32, tag="u2")
                    nc.vector.tensor_add(u2[:], agg[:, sl], ps2[:])
                    h1 = php.tile([128, 512], FP16, tag="h1")
                    nc.scalar.activation(h1[:], u2[:], AF.Relu, bias=c1t[:, 0:1])
                    nc.sync.dma_start(h1loc[:, sl], h1[:])
                    zt = ztp.tile([128, 4, 128], FP16, tag="zt")
                    for b in range(4):
                        ps = ps1p.tile([128, 512], F32, tag="ps1")
                        nc.tensor.matmul(ps[:, 0:128], lhsT=h1[:, b * 128:(b + 1) * 128],
                                         rhs=w2l[:], start=True, stop=True)
                        nc.scalar.copy(zt[:, b, :], ps[:, 0:128])
                    nc.sync.dma_start(
                        z2loc[w * 512:(w + 1) * 512, :].rearrange(
                            "(b p) f -> p b f", p=128), zt[:])

                # ---- sender gather layer 2 ----
                for q in range(ACELL // 1024):
                    st = ztp.tile([128, 8, 128], FP16, tag="st")
                    nc.gpsimd.dma_gather(
                        out_ap=st[:], in_ap=z2loc[:],
                        idxs_ap=sidx[:, q * 64:(q + 1) * 64],
                        num_idxs=1024, num_idxs_reg=1024, elem_size=128,
                        queue_num=q % 4)
                    nc.sync.dma_start(
                        a2a_in[q * 1024:(q + 1) * 1024, :].rearrange(
                            "(t p) f -> p t f", p=128), st[:])

                nc.gpsimd.collective_compute(
                    "AllToAll", mybir.AluOpType.bypass,
                    replica_groups=[list(range(C))],
                    ins=[a2a_in[:].opt()], outs=[halo[:].opt()])

                aggregate()

                # ---- phase G ----
                for w in range(NW):
                    sl = slice(w * 512, (w + 1) * 512)
                    h1w = xsp.tile([128, 512], FP16, tag="xa")
                    nc.sync.dma_start(h1w[:], h1loc[:, sl])
                    ps2 = ps1p.tile([128, 512], F32, tag="ps1")
                    nc.tensor.matmul(ps2[:], lhsT=w2r[:], rhs=h1w[:],
                                     start=True, stop=True)
                    u2 = php.tile([128, 512], F32, tag="u2")
                    nc.vector.tensor_add(u2[:], agg[:, sl], ps2[:])
                    h2 = php.tile([128, 512], FP16, tag="h2")
                    nc.scalar.activation(h2[:], u2[:], AF.Relu, bias=c2t[:, 0:1])
                    nc.sync.dma_start(hout_d[:, sl], h2[:])
    nc.compile()
    return nc


# ---------------------------------------------------------------------------
# PJRT runner (sharded over the 8 cores)
# ---------------------------------------------------------------------------

class _SpmdRunner:
    def __init__(self, nc, n_cores=C):
        import jax
        from jax.sharding import Mesh, PartitionSpec
        from jax.experimental.shard_map import shard_map
        import concourse.mybir as mybir
        from concourse.bass2jax import (_bass_exec_p, install_neuronx_cc_hook,
                                        partition_id_tensor)
        self.jax = jax
        install_neuronx_cc_hook()
        self.n_cores = n_cores
        partition_name = nc.partition_id_tensor.name if nc.partition_id_tensor else None
        in_names, out_names, out_avals, zero_outs = [], [], [], []
        for alloc in nc.m.functions[0].allocations:
            if not isinstance(alloc, mybir.MemoryLocationSet):
                continue
            name = alloc.memorylocations[0].name
            if alloc.kind == "ExternalInput":
                if name != partition_name:
                    in_names.append(name)
            elif alloc.kind == "ExternalOutput":
                shape = tuple(alloc.tensor_shape)
                dtype = mybir.dt.np(alloc.dtype)
                out_names.append(name)
                out_avals.append(jax.core.ShapedArray(shape, dtype))
                zero_outs.append(np.zeros(shape, dtype))
        self.in_names, self.out_names = in_names, out_names
        self.out_avals, self.zero_outs = out_avals, zero_outs
        n_params = len(in_names)
        all_in_names = list(in_names) + list(out_names)
        if partition_name is not None:
            all_in_names.append(partition_name)

        def _body(*args):
            operands = list(args)
            if partition_name is not None:
                operands.append(partition_id_tensor())
            outs = _bass_exec_p.bind(
                *operands,
                out_avals=tuple(out_avals),
                in_names=tuple(all_in_names),
                out_names=tuple(out_names),
                lowering_input_output_aliases=(),
                sim_require_finite=False,
                sim_require_nnan=False,
                nc=nc,
            )
            return tuple(outs)

        devices = jax.devices()[:n_cores]
        self.mesh = Mesh(np.asarray(devices), ("core",))
        in_specs = (PartitionSpec("core"),) * (n_params + len(out_names))
        out_specs = (PartitionSpec("core"),) * len(out_names)
        self.fn = jax.jit(
            shard_map(_body, mesh=self.mesh, in_specs=in_specs,
                      out_specs=out_specs, check_rep=False),
            keep_unused=True,
        )
        self.PartitionSpec = PartitionSpec

    def upload(self, in_maps):
        jax = self.jax
        n = self.n_cores
        per_core = [[np.asarray(m[name]) for name in self.in_names] for m in in_maps]
        concat_in = [np.concatenate([per_core[c][i] for c in range(n)], axis=0)
                     for i in range(len(self.in_names))]
        concat_zeros = [np.zeros((n * z.shape[0], *z.shape[1:]), z.dtype)
                        for z in self.zero_outs]
        sharding = jax.sharding.NamedSharding(self.mesh, self.PartitionSpec("core"))
        self.args = [jax.device_put(a, sharding) for a in concat_in + concat_zeros]

    def call_outputs(self):
        jax = self.jax
        outs = self.fn(*self.args)
        jax.block_until_ready(outs)
        n = self.n_cores
        return [
            {name: np.asarray(outs[i]).reshape(n, *self.out_avals[i].shape)[c]
             for i, name in enumerate(self.out_names)}
            for c in range(n)
        ]


def _prepare(edge_index, nreps=1):
    key = (hash(edge_index.tobytes()), nreps)
    if key in _cache:
        return _cache[key]
    g = _preprocess_graph(edge_index)
    nc = _build_kernel(g, nreps=nreps)
    runner = _SpmdRunner(nc)
    _cache[key] = (g, runner)
    return g, runner


def kernel(**inputs):
    edge_index = np.asarray(inputs['edge_index'])
    g, runner = _prepare(edge_index, nreps=1)
    in_maps = _make_in_maps(inputs, g)
    runner.upload(in_maps)
    outs = runner.call_outputs()
    return _assemble_output(outs, g)


# revision 3
# speedup vs baseline: 1.1161x; 1.1161x over previous
"""GraphSAGE 2-layer on 8 trn2 cores — v4: window-bucketed per-edge A2A halo.

Like v3 (per-edge AllToAll cells, one-hot scatter, no receiver gathers), but
every cell is subdivided into NWIN window-buckets of SHARED size BKT[w]
(max over all (core, shard) pairs, 128-aligned).  The receiver then works
window-major: for window w it streams the 8 cells' w-buckets (plain DMAs),
scatter-matmuls every 128-row tile into ONE accumulating PSUM window, and
flushes with a single tensor_copy into an fp16 agg.  Piece count == tile
count (no cross-core union), no flush adds, no agg memset.
Pad rows are masked by all-zero one-hot columns (dcmp == -1).
"""
import sys
sys.path.insert(0, '/opt/trn_rl_repo')

import numpy as np

N = 200000
E = 600000
C = 8
P = N // C
IN_DIM = 130
HID = 128
BN_EPS = 1e-5
WIN = 512
SL = 25088
NWIN = SL // WIN           # 49
F16 = np.float16

_cache = {}


def _round_up(x, m):
    return (x + m - 1) // m * m


def _wrap16(flat):
    s = len(flat)
    blk = np.asarray(flat, np.int16).reshape(s // 16, 16).T
    return np.tile(blk, (8, 1))


def _preprocess_graph(edge_index):
    src = edge_index[0].astype(np.int64)
    dst = edge_index[1].astype(np.int64)
    owner = dst // P
    dloc = dst - owner * P
    jsh = src // P
    wnd = dloc // WIN
    deg = np.bincount(dst, minlength=N)
    invd = (1.0 / np.maximum(deg, 1)).astype(np.float32)

    # bucket sizes shared across every (core, shard) pair
    cnt = np.zeros((C, C, NWIN), np.int64)
    np.add.at(cnt, (owner, jsh, wnd), 1)
    BKT = ((cnt.max(axis=(0, 1)) + 127) // 128) * 128      # [NWIN]
    OFF = np.concatenate(([0], np.cumsum(BKT)))            # [NWIN+1]
    CELLJ = int(OFF[-1])
    ACELL = C * CELLJ

    # order edges by (owner, shard, window, dloc); rank within run
    eorder = np.lexsort((dloc, wnd, jsh, owner))
    o_s, j_s, w_s, d_s, s_s = (owner[eorder], jsh[eorder], wnd[eorder],
                               dloc[eorder], src[eorder])
    key = (o_s * C + j_s) * NWIN + w_s
    bounds = np.searchsorted(key, np.arange(C * C * NWIN + 1))

    # per-halo-row tables (receiver view, core c)
    dcmp = np.full((C, ACELL), -1.0, np.float32)
    dinv = np.zeros((C, ACELL), np.float32)
    # sender view (core j): a2a_in row -> global src / src slot
    sendsrc = np.full((C, ACELL), -1, np.int64)
    sendflat = np.full((C, ACELL), SL, np.int64)
    for c in range(C):
        for j in range(C):
            for w in range(NWIN):
                k = (c * C + j) * NWIN + w
                a, b = bounds[k], bounds[k + 1]
                n = b - a
                base = j * CELLJ + OFF[w]
                dcmp[c, base:base + n] = d_s[a:b] - w * WIN
                dinv[c, base:base + n] = invd[c * P + d_s[a:b]]
                sbase = c * CELLJ + OFF[w]
                sendsrc[j, sbase:sbase + n] = s_s[a:b]
                sendflat[j, sbase:sbase + n] = s_s[a:b] - j * P
    sendidx = np.stack([_wrap16(sendflat[j]) for j in range(C)])

    # pieces: window-major, (w, j, t); NP = sum BKT/128 * C
    NP = int(BKT.sum() // 128) * C
    dcmp_in = np.empty((C, 128, NP), np.float32)
    dinv_in = np.empty((C, 128, NP), np.float32)
    pc = 0
    piece_plan = []            # (w, j, t, pc)
    for w in range(NWIN):
        for j in range(C):
            for t in range(int(BKT[w]) // 128):
                base = j * CELLJ + OFF[w] + t * 128
                dcmp_in[:, :, pc] = dcmp[:, base:base + 128]
                dinv_in[:, :, pc] = dinv[:, base:base + 128]
                piece_plan.append((w, j, t, pc))
                pc += 1
    assert pc == NP

    return dict(CELLJ=CELLJ, ACELL=ACELL, NP=NP, BKT=BKT.astype(int),
                OFF=OFF.astype(int), piece_plan=piece_plan,
                dcmp_in=dcmp_in, dinv_in=dinv_in,
                sendsrc=sendsrc, sendflat=sendflat, sendidx=sendidx)


def _fold_weights(Wl, bl, Wr, g, be, rm, rv):
    s = (np.asarray(g) / np.sqrt(np.asarray(rv) + BN_EPS)).astype(np.float32)
    Wl_f = (np.asarray(Wl) * s[None, :]).astype(np.float32)
    Wr_f = (np.asarray(Wr) * s[None, :]).astype(np.float32)
    c = ((np.asarray(bl) - np.asarray(rm)) * s + np.asarray(be)).astype(np.float32)
    return Wl_f, Wr_f, c


def _make_in_maps(inputs, g):
    x = np.asarray(inputs['x'], np.float32)
    W1l, W1r, c1 = _fold_weights(inputs['W1_l'], inputs['b1_l'], inputs['W1_r'],
                                 inputs['g1'], inputs['be1'], inputs['rm1'],
                                 inputs['rv1'])
    W2l, W2r, c2 = _fold_weights(inputs['W2_l'], inputs['b2_l'], inputs['W2_r'],
                                 inputs['g2'], inputs['be2'], inputs['rm2'],
                                 inputs['rv2'])
    ACELL = g['ACELL']
    shared = {
        'W1la': W1l[0:128].astype(F16), 'W1lb': W1l[128:130].astype(F16),
        'W1ra': W1r[0:128].astype(F16), 'W1rb': W1r[128:130].astype(F16),
        'W2l': W2l.astype(F16), 'W2r': W2r.astype(F16),
        'c1': c1.reshape(128, 1), 'c2': c2.reshape(128, 1),
    }
    in_maps = []
    for c in range(C):
        m = dict(shared)
        xn = np.zeros((IN_DIM, SL), np.float32)
        xn[:, :P] = x[c * P:(c + 1) * P].T
        m['xna'] = xn[0:128].astype(F16)
        m['xnb'] = xn[128:130].astype(F16)
        xc = np.zeros((IN_DIM, ACELL), np.float32)
        sel = g['sendsrc'][c] >= 0
        xc[:, sel] = x[g['sendsrc'][c][sel]].T
        m['xca'] = xc[0:128].astype(F16)
        m['xcb'] = xc[128:130].astype(F16)
        m['dcmp'] = g['dcmp_in'][c]
        m['dinv'] = g['dinv_in'][c]
        m['iota'] = np.tile(np.arange(WIN, dtype=np.float32), (128, 1))
        m['sendidx'] = g['sendidx'][c]
        in_maps.append(m)
    return in_maps


def _assemble_output(outs, g):
    h2 = np.empty((N, HID), np.float32)
    for c in range(C):
        h2[c * P:(c + 1) * P] = outs[c]['hout'][:, :P].T.astype(np.float32)
    return h2


# ---------------------------------------------------------------------------
# numpy emulation
# ---------------------------------------------------------------------------

def _emulate_full(in_maps, g):
    CELLJ = g['CELLJ']

    def f16(a):
        return a.astype(F16).astype(np.float32)

    W1la = in_maps[0]['W1la'].astype(np.float32)
    W1lb = in_maps[0]['W1lb'].astype(np.float32)
    W1ra = in_maps[0]['W1ra'].astype(np.float32)
    W1rb = in_maps[0]['W1rb'].astype(np.float32)
    W2l = in_maps[0]['W2l'].astype(np.float32)
    W2r = in_maps[0]['W2r'].astype(np.float32)
    c1 = in_maps[0]['c1'].ravel()
    c2 = in_maps[0]['c2'].ravel()

    def aggregate(zin_rows):
        halo = [np.concatenate([zin_rows[j][c * CELLJ:(c + 1) * CELLJ]
                                for j in range(C)]) for c in range(C)]
        aggs = []
        iota = np.arange(WIN, dtype=np.float32)
        for c in range(C):
            agg = np.zeros((128, SL), np.float32)
            dc = in_maps[c]['dcmp'].astype(np.float32)
            di = in_maps[c]['dinv'].astype(np.float32)
            for (w, j, t, pc) in g['piece_plan']:
                base = j * CELLJ + g['OFF'][w] + t * 128
                tile = halo[c][base:base + 128]
                oh = f16((iota[None, :] == dc[:, pc:pc + 1]) * di[:, pc:pc + 1])
                agg[:, w * WIN:(w + 1) * WIN] += tile.T @ oh
            aggs.append(f16(agg))
        return aggs

    zcells = []
    for c in range(C):
        xca = in_maps[c]['xca'].astype(np.float32)
        xcb = in_maps[c]['xcb'].astype(np.float32)
        zcells.append(f16(xca.T @ W1la + xcb.T @ W1lb))
    aggs = aggregate(zcells)
    h1s = []
    z2locs = []
    for c in range(C):
        xna = in_maps[c]['xna'].astype(np.float32)
        xnb = in_maps[c]['xnb'].astype(np.float32)
        r1 = W1ra.T @ xna + W1rb.T @ xnb
        h1 = f16(np.maximum(aggs[c] + r1 + c1[:, None], 0.0))
        h1s.append(h1)
        z2 = np.concatenate([f16(h1.T @ W2l), np.zeros((16, 128), np.float32)])
        z2locs.append(z2)
    zcells2 = [z2locs[c][g['sendflat'][c]] for c in range(C)]
    aggs2 = aggregate(zcells2)
    outs = []
    for c in range(C):
        r2 = W2r.T @ h1s[c]
        h2 = f16(np.maximum(aggs2[c] + r2 + c2[:, None], 0.0))
        outs.append({'hout': h2.astype(F16)})
    return outs


# ---------------------------------------------------------------------------
# bass kernel
# ---------------------------------------------------------------------------

def _build_kernel(g, nreps=1):
    import concourse.bacc as bacc
    import concourse.tile as tile
    import concourse.mybir as mybir

    F32 = mybir.dt.float32
    FP16 = mybir.dt.float16
    I16 = mybir.dt.int16
    AF = mybir.ActivationFunctionType
    ALU = mybir.AluOpType

    ACELL, CELLJ, NP = g['ACELL'], g['CELLJ'], g['NP']
    BKT, OFF = g['BKT'], g['OFF']
    NKC = ACELL // 512
    NW = SL // 512
    maxbt = int(max(BKT)) // 128

    nc = bacc.Bacc("TRN2", target_bir_lowering=False, debug=False, num_devices=C,
                   num_swdge_queues=4)

    xna_d = nc.dram_tensor("xna", [128, SL], FP16, kind="ExternalInput")
    xnb_d = nc.dram_tensor("xnb", [2, SL], FP16, kind="ExternalInput")
    xca_d = nc.dram_tensor("xca", [128, ACELL], FP16, kind="ExternalInput")
    xcb_d = nc.dram_tensor("xcb", [2, ACELL], FP16, kind="ExternalInput")
    W1la_d = nc.dram_tensor("W1la", [128, 128], FP16, kind="ExternalInput")
    W1lb_d = nc.dram_tensor("W1lb", [2, 128], FP16, kind="ExternalInput")
    W1ra_d = nc.dram_tensor("W1ra", [128, 128], FP16, kind="ExternalInput")
    W1rb_d = nc.dram_tensor("W1rb", [2, 128], FP16, kind="ExternalInput")
    W2l_d = nc.dram_tensor("W2l", [128, 128], FP16, kind="ExternalInput")
    W2r_d = nc.dram_tensor("W2r", [128, 128], FP16, kind="ExternalInput")
    c1_d = nc.dram_tensor("c1", [128, 1], F32, kind="ExternalInput")
    c2_d = nc.dram_tensor("c2", [128, 1], F32, kind="ExternalInput")
    dcmp_d = nc.dram_tensor("dcmp", [128, NP], F32, kind="ExternalInput")
    dinv_d = nc.dram_tensor("dinv", [128, NP], F32, kind="ExternalInput")
    iota_d = nc.dram_tensor("iota", [128, WIN], F32, kind="ExternalInput")
    sidx_d = nc.dram_tensor("sendidx", [128, ACELL // 16], I16, kind="ExternalInput")
    hout_d = nc.dram_tensor("hout", [128, SL], FP16, kind="ExternalOutput")

    a2a_in = nc.dram_tensor("a2ain", [ACELL, 128], FP16)
    halo = nc.dram_tensor("halo", [ACELL, 128], FP16)
    z2loc = nc.dram_tensor("z2loc", [SL + 16, 128], FP16)

    with tile.TileContext(nc) as tc:
        with (
            tc.tile_pool(name="const", bufs=1) as cons,
            tc.tile_pool(name="big", bufs=1) as bigp,
            tc.tile_pool(name="xs", bufs=3) as xsp,
            tc.tile_pool(name="zt", bufs=3) as ztp,
            tc.tile_pool(name="hc", bufs=4) as hcp,
            tc.tile_pool(name="oh", bufs=4) as ohp,
            tc.tile_pool(name="ph", bufs=3) as php,
            tc.tile_pool(name="agps", bufs=3, space="PSUM") as agps,
            tc.tile_pool(name="ps1", bufs=3, space="PSUM") as ps1p,
        ):
            w1la = cons.tile([128, 128], FP16)
            w1lb = cons.tile([2, 128], FP16)
            w1ra = cons.tile([128, 128], FP16)
            w1rb = cons.tile([2, 128], FP16)
            w2l = cons.tile([128, 128], FP16)
            w2r = cons.tile([128, 128], FP16)
            c1t = cons.tile([128, 1], F32)
            c2t = cons.tile([128, 1], F32)
            dcmp_t = cons.tile([128, NP], F32)
            dinv_t = cons.tile([128, NP], F32)
            iota_t = cons.tile([128, WIN], F32)
            sidx = cons.tile([128, ACELL // 16], I16)
            for sb, dr in ((w1la, W1la_d), (w1lb, W1lb_d), (w1ra, W1ra_d),
                           (w1rb, W1rb_d), (w2l, W2l_d), (w2r, W2r_d),
                           (c1t, c1_d), (c2t, c2_d), (dcmp_t, dcmp_d),
                           (dinv_t, dinv_d), (iota_t, iota_d), (sidx, sidx_d)):
                nc.sync.dma_start(sb[:], dr[:])
            zrow = cons.tile([1, 128], FP16)
            nc.vector.memset(zrow[:], 0.0)
            nc.sync.dma_start(z2loc[SL:SL + 1, :], zrow[:])

            agg = bigp.tile([128, SL], FP16)
            h1buf = bigp.tile([128, SL], FP16)

            def aggregate():
                pc = 0
                for w in range(NWIN):
                    ps = agps.tile([128, WIN], F32, tag="agps")
                    first = True
                    for j in range(C):
                        nt = int(BKT[w]) // 128
                        hch = hcp.tile([128, maxbt, 128], FP16, tag="hch")
                        base = j * CELLJ + int(OFF[w])
                        nc.sync.dma_start(
                            hch[:, 0:nt, :],
                            halo[base:base + nt * 128, :].rearrange(
                                "(b p) f -> p b f", p=128))
                        for t in range(nt):
                            oh = ohp.tile([128, WIN], FP16, tag="oh")
                            nc.vector.tensor_scalar(
                                oh[:], iota_t[:],
                                dcmp_t[:, pc:pc + 1], dinv_t[:, pc:pc + 1],
                                op0=ALU.is_equal, op1=ALU.mult)
                            nc.tensor.matmul(ps[:], lhsT=hch[:, t, :], rhs=oh[:],
                                             start=first, stop=False,
                                             skip_group_check=True)
                            first = False
                            pc += 1
                    nc.vector.tensor_copy(agg[:, w * WIN:(w + 1) * WIN], ps[:])
                assert pc == NP

            for rep in range(nreps):
                # ---- phase A: z1 per edge-slot -> a2a_in ----
                for k in range(NKC):
                    xa = xsp.tile([128, 512], FP16, tag="xa")
                    xb = xsp.tile([2, 512], FP16, tag="xb")
                    nc.sync.dma_start(xa[:], xca_d[:, k * 512:(k + 1) * 512])
                    nc.sync.dma_start(xb[:], xcb_d[:, k * 512:(k + 1) * 512])
                    zt = ztp.tile([128, 4, 128], FP16, tag="zt")
                    ps = ps1p.tile([128, 512], F32, tag="ps1")
                    for b in range(4):
                        nc.tensor.matmul(ps[:, b * 128:(b + 1) * 128],
                                         lhsT=xa[:, b * 128:(b + 1) * 128],
                                         rhs=w1la[:], start=True, stop=False,
                                         skip_group_check=True)
                        nc.tensor.matmul(ps[:, b * 128:(b + 1) * 128],
                                         lhsT=xb[:, b * 128:(b + 1) * 128],
                                         rhs=w1lb[:], start=False, stop=True,
                                         skip_group_check=True)
                    nc.scalar.copy(zt[:].rearrange("p b f -> p (b f)"), ps[:])
                    nc.sync.dma_start(
                        a2a_in[k * 512:(k + 1) * 512, :].rearrange(
                            "(b p) f -> p b f", p=128), zt[:])

                nc.gpsimd.collective_compute(
                    "AllToAll", mybir.AluOpType.bypass,
                    replica_groups=[list(range(C))],
                    ins=[a2a_in[:].opt()], outs=[halo[:].opt()])

                aggregate()

                # ---- phase D ----
                for w in range(NW):
                    sl = slice(w * 512, (w + 1) * 512)
                    xa = xsp.tile([128, 512], FP16, tag="xa")
                    xb = xsp.tile([2, 512], FP16, tag="xb")
                    nc.sync.dma_start(xa[:], xna_d[:, sl])
                    nc.sync.dma_start(xb[:], xnb_d[:, sl])
                    ps2 = ps1p.tile([128, 512], F32, tag="ps1")
                    nc.tensor.matmul(ps2[:], lhsT=w1ra[:], rhs=xa[:],
                                     start=True, stop=False)
                    nc.tensor.matmul(ps2[:], lhsT=w1rb[:], rhs=xb[:],
                                     start=False, stop=True)
                    u2 = php.tile([128, 512], F32, tag="u2")
                    nc.vector.tensor_add(u2[:], agg[:, sl], ps2[:])
                    h1 = h1buf[:, sl]
                    nc.scalar.activation(h1, u2[:], AF.Relu, bias=c1t[:, 0:1])
                    zt = ztp.tile([128, 4, 128], FP16, tag="zt")
                    ps = ps1p.tile([128, 512], F32, tag="ps1")
                    for b in range(4):
                        nc.tensor.matmul(ps[:, b * 128:(b + 1) * 128],
                                         lhsT=h1buf[:, w * 512 + b * 128:
                                                    w * 512 + (b + 1) * 128],
                                         rhs=w2l[:], start=True, stop=True,
                                         skip_group_check=True)
                    nc.scalar.copy(zt[:].rearrange("p b f -> p (b f)"), ps[:])
                    nc.sync.dma_start(
                        z2loc[w * 512:(w + 1) * 512, :].rearrange(
                            "(b p) f -> p b f", p=128), zt[:])

                # ---- sender gather layer 2 ----
                for q in range(ACELL // 1024):
                    st = ztp.tile([128, 8, 128], FP16, tag="st")
                    nc.gpsimd.dma_gather(
                        out_ap=st[:], in_ap=z2loc[:],
                        idxs_ap=sidx[:, q * 64:(q + 1) * 64],
                        num_idxs=1024, num_idxs_reg=1024, elem_size=128,
                        queue_num=q % 4)
                    nc.sync.dma_start(
                        a2a_in[q * 1024:(q + 1) * 1024, :].rearrange(
                            "(t p) f -> p t f", p=128), st[:])

                nc.gpsimd.collective_compute(
                    "AllToAll", mybir.AluOpType.bypass,
                    replica_groups=[list(range(C))],
                    ins=[a2a_in[:].opt()], outs=[halo[:].opt()])

                aggregate()

                # ---- phase G ----
                for w in range(NW):
                    sl = slice(w * 512, (w + 1) * 512)
                    ps2 = ps1p.tile([128, 512], F32, tag="ps1")
                    nc.tensor.matmul(ps2[:], lhsT=w2r[:], rhs=h1buf[:, sl],
                                     start=True, stop=True)
                    u2 = php.tile([128, 512], F32, tag="u2")
                    nc.vector.tensor_add(u2[:], agg[:, sl], ps2[:])
                    h2 = php.tile([128, 512], FP16, tag="h2")
                    nc.scalar.activation(h2[:], u2[:], AF.Relu, bias=c2t[:, 0:1])
                    nc.sync.dma_start(hout_d[:, sl], h2[:])
    nc.compile()
    return nc


# ---------------------------------------------------------------------------
# PJRT runner (sharded over the 8 cores)
# ---------------------------------------------------------------------------

class _SpmdRunner:
    def __init__(self, nc, n_cores=C):
        import jax
        from jax.sharding import Mesh, PartitionSpec
        from jax.experimental.shard_map import shard_map
        import concourse.mybir as mybir
        from concourse.bass2jax import (_bass_exec_p, install_neuronx_cc_hook,
                                        partition_id_tensor)
        self.jax = jax
        install_neuronx_cc_hook()
        self.n_cores = n_cores
        partition_name = nc.partition_id_tensor.name if nc.partition_id_tensor else None
        in_names, out_names, out_avals, zero_outs = [], [], [], []
        for alloc in nc.m.functions[0].allocations:
            if not isinstance(alloc, mybir.MemoryLocationSet):
                continue
            name = alloc.memorylocations[0].name
            if alloc.kind == "ExternalInput":
                if name != partition_name:
                    in_names.append(name)
            elif alloc.kind == "ExternalOutput":
                shape = tuple(alloc.tensor_shape)
                dtype = mybir.dt.np(alloc.dtype)
                out_names.append(name)
                out_avals.append(jax.core.ShapedArray(shape, dtype))
                zero_outs.append(np.zeros(shape, dtype))
        self.in_names, self.out_names = in_names, out_names
        self.out_avals, self.zero_outs = out_avals, zero_outs
        n_params = len(in_names)
        all_in_names = list(in_names) + list(out_names)
        if partition_name is not None:
            all_in_names.append(partition_name)

        def _body(*args):
            operands = list(args)
            if partition_name is not None:
                operands.append(partition_id_tensor())
            outs = _bass_exec_p.bind(
                *operands,
                out_avals=tuple(out_avals),
                in_names=tuple(all_in_names),
                out_names=tuple(out_names),
                lowering_input_output_aliases=(),
                sim_require_finite=False,
                sim_require_nnan=False,
                nc=nc,
            )
            return tuple(outs)

        devices = jax.devices()[:n_cores]
        self.mesh = Mesh(np.asarray(devices), ("core",))
        in_specs = (PartitionSpec("core"),) * (n_params + len(out_names))
        out_specs = (PartitionSpec("core"),) * len(out_names)
        self.fn = jax.jit(
            shard_map(_body, mesh=self.mesh, in_specs=in_specs,
                      out_specs=out_specs, check_rep=False),
            keep_unused=True,
        )
        self.PartitionSpec = PartitionSpec

    def upload(self, in_maps):
        jax = self.jax
        n = self.n_cores
        per_core = [[np.asarray(m[name]) for name in self.in_names] for m in in_maps]
        concat_in = [np.concatenate([per_core[c][i] for c in range(n)], axis=0)
                     for i in range(len(self.in_names))]
        concat_zeros = [np.zeros((n * z.shape[0], *z.shape[1:]), z.dtype)
                        for z in self.zero_outs]
        sharding = jax.sharding.NamedSharding(self.mesh, self.PartitionSpec("core"))
        self.args = [jax.device_put(a, sharding) for a in concat_in + concat_zeros]

    def call_outputs(self):
        jax = self.jax
        outs = self.fn(*self.args)
        jax.block_until_ready(outs)
        n = self.n_cores
        return [
            {name: np.asarray(outs[i]).reshape(n, *self.out_avals[i].shape)[c]
             for i, name in enumerate(self.out_names)}
            for c in range(n)
        ]


def _prepare(edge_index, nreps=1):
    key = (hash(edge_index.tobytes()), nreps)
    if key in _cache:
        return _cache[key]
    g = _preprocess_graph(edge_index)
    nc = _build_kernel(g, nreps=nreps)
    runner = _SpmdRunner(nc)
    _cache[key] = (g, runner)
    return g, runner


def kernel(**inputs):
    edge_index = np.asarray(inputs['edge_index'])
    g, runner = _prepare(edge_index, nreps=1)
    in_maps = _make_in_maps(inputs, g)
    runner.upload(in_maps)
    outs = runner.call_outputs()
    return _assemble_output(outs, g)


# revision 4
# speedup vs baseline: 1.2442x; 1.1148x over previous
"""GraphSAGE 2-layer on 8 trn2 cores — v4: window-bucketed per-edge A2A halo.

Like v3 (per-edge AllToAll cells, one-hot scatter, no receiver gathers), but
every cell is subdivided into NWIN window-buckets of SHARED size BKT[w]
(max over all (core, shard) pairs, 128-aligned).  The receiver then works
window-major: for window w it streams the 8 cells' w-buckets (plain DMAs),
scatter-matmuls every 128-row tile into ONE accumulating PSUM window, and
flushes with a single tensor_copy into an fp16 agg.  Piece count == tile
count (no cross-core union), no flush adds, no agg memset.
Pad rows are masked by all-zero one-hot columns (dcmp == -1).
"""
import sys
sys.path.insert(0, '/opt/trn_rl_repo')

import numpy as np

N = 200000
E = 600000
C = 8
P = N // C
IN_DIM = 130
HID = 128
BN_EPS = 1e-5
WIN = 512
SL = 25088
NWIN = SL // WIN           # 49
F16 = np.float16

_cache = {}


def _round_up(x, m):
    return (x + m - 1) // m * m


def _wrap16(flat):
    s = len(flat)
    blk = np.asarray(flat, np.int16).reshape(s // 16, 16).T
    return np.tile(blk, (8, 1))


def _preprocess_graph(edge_index):
    src = edge_index[0].astype(np.int64)
    dst = edge_index[1].astype(np.int64)
    owner = dst // P
    dloc = dst - owner * P
    jsh = src // P
    wnd = dloc // WIN
    deg = np.bincount(dst, minlength=N)
    invd = (1.0 / np.maximum(deg, 1)).astype(np.float32)

    # bucket sizes shared across every (core, shard) pair
    cnt = np.zeros((C, C, NWIN), np.int64)
    np.add.at(cnt, (owner, jsh, wnd), 1)
    BKT = ((cnt.max(axis=(0, 1)) + 127) // 128) * 128      # [NWIN]
    OFF = np.concatenate(([0], np.cumsum(BKT)))            # [NWIN+1]
    CELLJ = int(OFF[-1])
    ACELL = C * CELLJ

    # order edges by (owner, shard, window, dloc); rank within run
    eorder = np.lexsort((dloc, wnd, jsh, owner))
    o_s, j_s, w_s, d_s, s_s = (owner[eorder], jsh[eorder], wnd[eorder],
                               dloc[eorder], src[eorder])
    key = (o_s * C + j_s) * NWIN + w_s
    bounds = np.searchsorted(key, np.arange(C * C * NWIN + 1))

    # per-halo-row tables (receiver view, core c)
    dcmp = np.full((C, ACELL), -1.0, np.float32)
    dinv = np.zeros((C, ACELL), np.float32)
    # sender view (core j): a2a_in row -> global src / src slot
    sendsrc = np.full((C, ACELL), -1, np.int64)
    sendflat = np.full((C, ACELL), SL, np.int64)
    for c in range(C):
        for j in range(C):
            for w in range(NWIN):
                k = (c * C + j) * NWIN + w
                a, b = bounds[k], bounds[k + 1]
                n = b - a
                base = j * CELLJ + OFF[w]
                dcmp[c, base:base + n] = d_s[a:b] - w * WIN
                dinv[c, base:base + n] = invd[c * P + d_s[a:b]]
                sbase = c * CELLJ + OFF[w]
                sendsrc[j, sbase:sbase + n] = s_s[a:b]
                sendflat[j, sbase:sbase + n] = s_s[a:b] - j * P
    sendidx = np.stack([_wrap16(sendflat[j]) for j in range(C)])

    # pieces: window-major, (w, j, t); NP = sum BKT/128 * C
    NP = int(BKT.sum() // 128) * C
    dcmp_in = np.empty((C, 128, NP), np.float32)
    dinv_in = np.empty((C, 128, NP), np.float32)
    pc = 0
    piece_plan = []            # (w, j, t, pc)
    for w in range(NWIN):
        for j in range(C):
            for t in range(int(BKT[w]) // 128):
                base = j * CELLJ + OFF[w] + t * 128
                dcmp_in[:, :, pc] = dcmp[:, base:base + 128]
                dinv_in[:, :, pc] = dinv[:, base:base + 128]
                piece_plan.append((w, j, t, pc))
                pc += 1
    assert pc == NP

    return dict(CELLJ=CELLJ, ACELL=ACELL, NP=NP, BKT=BKT.astype(int),
                OFF=OFF.astype(int), piece_plan=piece_plan,
                dcmp_in=dcmp_in, dinv_in=dinv_in,
                sendsrc=sendsrc, sendflat=sendflat, sendidx=sendidx)


def _fold_weights(Wl, bl, Wr, g, be, rm, rv):
    s = (np.asarray(g) / np.sqrt(np.asarray(rv) + BN_EPS)).astype(np.float32)
    Wl_f = (np.asarray(Wl) * s[None, :]).astype(np.float32)
    Wr_f = (np.asarray(Wr) * s[None, :]).astype(np.float32)
    c = ((np.asarray(bl) - np.asarray(rm)) * s + np.asarray(be)).astype(np.float32)
    return Wl_f, Wr_f, c


def _make_in_maps(inputs, g):
    x = np.asarray(inputs['x'], np.float32)
    W1l, W1r, c1 = _fold_weights(inputs['W1_l'], inputs['b1_l'], inputs['W1_r'],
                                 inputs['g1'], inputs['be1'], inputs['rm1'],
                                 inputs['rv1'])
    W2l, W2r, c2 = _fold_weights(inputs['W2_l'], inputs['b2_l'], inputs['W2_r'],
                                 inputs['g2'], inputs['be2'], inputs['rm2'],
                                 inputs['rv2'])
    ACELL = g['ACELL']
    shared = {
        'W1la': W1l[0:128].astype(F16), 'W1lb': W1l[128:130].astype(F16),
        'W1ra': W1r[0:128].astype(F16), 'W1rb': W1r[128:130].astype(F16),
        'W2l': W2l.astype(F16), 'W2r': W2r.astype(F16),
        'c1': c1.reshape(128, 1), 'c2': c2.reshape(128, 1),
    }
    in_maps = []
    for c in range(C):
        m = dict(shared)
        xn = np.zeros((IN_DIM, SL), np.float32)
        xn[:, :P] = x[c * P:(c + 1) * P].T
        m['xna'] = xn[0:128].astype(F16)
        m['xnb'] = xn[128:130].astype(F16)
        xc = np.zeros((IN_DIM, ACELL), np.float32)
        sel = g['sendsrc'][c] >= 0
        xc[:, sel] = x[g['sendsrc'][c][sel]].T
        m['xca'] = xc[0:128].astype(F16)
        m['xcb'] = xc[128:130].astype(F16)
        m['dcmp'] = g['dcmp_in'][c]
        m['dinv'] = g['dinv_in'][c]
        m['iota'] = np.tile(np.arange(WIN, dtype=np.float32), (128, 1))
        m['sendidx'] = g['sendidx'][c]
        in_maps.append(m)
    return in_maps


def _assemble_output(outs, g):
    h2 = np.empty((N, HID), np.float32)
    for c in range(C):
        h2[c * P:(c + 1) * P] = outs[c]['hout'][:, :P].T.astype(np.float32)
    return h2


# ---------------------------------------------------------------------------
# numpy emulation
# ---------------------------------------------------------------------------

def _emulate_full(in_maps, g):
    CELLJ = g['CELLJ']

    def f16(a):
        return a.astype(F16).astype(np.float32)

    W1la = in_maps[0]['W1la'].astype(np.float32)
    W1lb = in_maps[0]['W1lb'].astype(np.float32)
    W1ra = in_maps[0]['W1ra'].astype(np.float32)
    W1rb = in_maps[0]['W1rb'].astype(np.float32)
    W2l = in_maps[0]['W2l'].astype(np.float32)
    W2r = in_maps[0]['W2r'].astype(np.float32)
    c1 = in_maps[0]['c1'].ravel()
    c2 = in_maps[0]['c2'].ravel()

    def aggregate(zin_rows):
        halo = [np.concatenate([zin_rows[j][c * CELLJ:(c + 1) * CELLJ]
                                for j in range(C)]) for c in range(C)]
        aggs = []
        iota = np.arange(WIN, dtype=np.float32)
        for c in range(C):
            agg = np.zeros((128, SL), np.float32)
            dc = in_maps[c]['dcmp'].astype(np.float32)
            di = in_maps[c]['dinv'].astype(np.float32)
            for (w, j, t, pc) in g['piece_plan']:
                base = j * CELLJ + g['OFF'][w] + t * 128
                tile = halo[c][base:base + 128]
                oh = f16((iota[None, :] == dc[:, pc:pc + 1]) * di[:, pc:pc + 1])
                agg[:, w * WIN:(w + 1) * WIN] += tile.T @ oh
            aggs.append(f16(agg))
        return aggs

    zcells = []
    for c in range(C):
        xca = in_maps[c]['xca'].astype(np.float32)
        xcb = in_maps[c]['xcb'].astype(np.float32)
        zcells.append(f16(xca.T @ W1la + xcb.T @ W1lb))
    aggs = aggregate(zcells)
    h1s = []
    z2locs = []
    for c in range(C):
        xna = in_maps[c]['xna'].astype(np.float32)
        xnb = in_maps[c]['xnb'].astype(np.float32)
        r1 = W1ra.T @ xna + W1rb.T @ xnb
        h1 = f16(np.maximum(aggs[c] + r1 + c1[:, None], 0.0))
        h1s.append(h1)
        z2 = np.concatenate([f16(h1.T @ W2l), np.zeros((16, 128), np.float32)])
        z2locs.append(z2)
    zcells2 = [z2locs[c][g['sendflat'][c]] for c in range(C)]
    aggs2 = aggregate(zcells2)
    outs = []
    for c in range(C):
        r2 = W2r.T @ h1s[c]
        h2 = f16(np.maximum(aggs2[c] + r2 + c2[:, None], 0.0))
        outs.append({'hout': h2.astype(F16)})
    return outs


# ---------------------------------------------------------------------------
# bass kernel
# ---------------------------------------------------------------------------

def _build_kernel(g, nreps=1):
    import concourse.bacc as bacc
    import concourse.tile as tile
    import concourse.mybir as mybir

    F32 = mybir.dt.float32
    FP16 = mybir.dt.float16
    I16 = mybir.dt.int16
    AF = mybir.ActivationFunctionType
    ALU = mybir.AluOpType

    ACELL, CELLJ, NP = g['ACELL'], g['CELLJ'], g['NP']
    BKT, OFF = g['BKT'], g['OFF']
    NKC = ACELL // 512
    NW = SL // 512
    maxbt = int(max(BKT)) // 128

    nc = bacc.Bacc("TRN2", target_bir_lowering=False, debug=False, num_devices=C,
                   num_swdge_queues=4)

    xna_d = nc.dram_tensor("xna", [128, SL], FP16, kind="ExternalInput")
    xnb_d = nc.dram_tensor("xnb", [2, SL], FP16, kind="ExternalInput")
    xca_d = nc.dram_tensor("xca", [128, ACELL], FP16, kind="ExternalInput")
    xcb_d = nc.dram_tensor("xcb", [2, ACELL], FP16, kind="ExternalInput")
    W1la_d = nc.dram_tensor("W1la", [128, 128], FP16, kind="ExternalInput")
    W1lb_d = nc.dram_tensor("W1lb", [2, 128], FP16, kind="ExternalInput")
    W1ra_d = nc.dram_tensor("W1ra", [128, 128], FP16, kind="ExternalInput")
    W1rb_d = nc.dram_tensor("W1rb", [2, 128], FP16, kind="ExternalInput")
    W2l_d = nc.dram_tensor("W2l", [128, 128], FP16, kind="ExternalInput")
    W2r_d = nc.dram_tensor("W2r", [128, 128], FP16, kind="ExternalInput")
    c1_d = nc.dram_tensor("c1", [128, 1], F32, kind="ExternalInput")
    c2_d = nc.dram_tensor("c2", [128, 1], F32, kind="ExternalInput")
    dcmp_d = nc.dram_tensor("dcmp", [128, NP], F32, kind="ExternalInput")
    dinv_d = nc.dram_tensor("dinv", [128, NP], F32, kind="ExternalInput")
    iota_d = nc.dram_tensor("iota", [128, WIN], F32, kind="ExternalInput")
    sidx_d = nc.dram_tensor("sendidx", [128, ACELL // 16], I16, kind="ExternalInput")
    hout_d = nc.dram_tensor("hout", [128, SL], FP16, kind="ExternalOutput")

    a2a_in = nc.dram_tensor("a2ain", [ACELL, 128], FP16)
    halo = nc.dram_tensor("halo", [ACELL, 128], FP16)
    z2loc = nc.dram_tensor("z2loc", [SL + 16, 128], FP16)

    with tile.TileContext(nc) as tc:
        with (
            tc.tile_pool(name="const", bufs=1) as cons,
            tc.tile_pool(name="big", bufs=1) as bigp,
            tc.tile_pool(name="xs", bufs=3) as xsp,
            tc.tile_pool(name="zt", bufs=3) as ztp,
            tc.tile_pool(name="hc", bufs=4) as hcp,
            tc.tile_pool(name="oh", bufs=4) as ohp,
            tc.tile_pool(name="ph", bufs=3) as php,
            tc.tile_pool(name="agps", bufs=3, space="PSUM") as agps,
            tc.tile_pool(name="ps1", bufs=3, space="PSUM") as ps1p,
        ):
            w1la = cons.tile([128, 128], FP16)
            w1lb = cons.tile([2, 128], FP16)
            w1ra = cons.tile([128, 128], FP16)
            w1rb = cons.tile([2, 128], FP16)
            w2l = cons.tile([128, 128], FP16)
            w2r = cons.tile([128, 128], FP16)
            c1t = cons.tile([128, 1], F32)
            c2t = cons.tile([128, 1], F32)
            dcmp_t = cons.tile([128, NP], F32)
            dinv_t = cons.tile([128, NP], F32)
            iota_t = cons.tile([128, WIN], F32)
            sidx = cons.tile([128, ACELL // 16], I16)
            for sb, dr in ((w1la, W1la_d), (w1lb, W1lb_d), (w1ra, W1ra_d),
                           (w1rb, W1rb_d), (w2l, W2l_d), (w2r, W2r_d),
                           (c1t, c1_d), (c2t, c2_d), (dcmp_t, dcmp_d),
                           (dinv_t, dinv_d), (iota_t, iota_d), (sidx, sidx_d)):
                nc.sync.dma_start(sb[:], dr[:])
            zrow = cons.tile([1, 128], FP16)
            nc.vector.memset(zrow[:], 0.0)
            nc.sync.dma_start(z2loc[SL:SL + 1, :], zrow[:])

            h1buf = bigp.tile([128, SL], FP16)

            def aggregate(layer):
                pc = 0
                for w in range(NWIN):
                    sl = slice(w * 512, (w + 1) * 512)
                    ps = agps.tile([128, WIN], F32, tag="agps")
                    first = True
                    for j in range(C):
                        nt = int(BKT[w]) // 128
                        hch = hcp.tile([128, maxbt, 128], FP16, tag="hch")
                        base = j * CELLJ + int(OFF[w])
                        nc.sync.dma_start(
                            hch[:, 0:nt, :],
                            halo[base:base + nt * 128, :].rearrange(
                                "(b p) f -> p b f", p=128))
                        for t in range(nt):
                            oh = ohp.tile([128, WIN], FP16, tag="oh")
                            nc.vector.tensor_scalar(
                                oh[:], iota_t[:],
                                dcmp_t[:, pc:pc + 1], dinv_t[:, pc:pc + 1],
                                op0=ALU.is_equal, op1=ALU.mult)
                            nc.tensor.matmul(ps[:], lhsT=hch[:, t, :], rhs=oh[:],
                                             start=first, stop=False,
                                             skip_group_check=True)
                            first = False
                            pc += 1
                    if layer == 1:
                        xa = xsp.tile([128, 512], FP16, tag="xa")
                        xb = xsp.tile([2, 512], FP16, tag="xb")
                        nc.sync.dma_start(xa[:], xna_d[:, sl])
                        nc.sync.dma_start(xb[:], xnb_d[:, sl])
                        nc.tensor.matmul(ps[:], lhsT=w1ra[:], rhs=xa[:],
                                         start=False, stop=False,
                                         skip_group_check=True)
                        nc.tensor.matmul(ps[:], lhsT=w1rb[:], rhs=xb[:],
                                         start=False, stop=True,
                                         skip_group_check=True)
                        nc.scalar.activation(h1buf[:, sl], ps[:], AF.Relu,
                                             bias=c1t[:, 0:1])
                        zt = ztp.tile([128, 4, 128], FP16, tag="zt")
                        psz = ps1p.tile([128, 512], F32, tag="ps1")
                        for b in range(4):
                            nc.tensor.matmul(psz[:, b * 128:(b + 1) * 128],
                                             lhsT=h1buf[:, w * 512 + b * 128:
                                                        w * 512 + (b + 1) * 128],
                                             rhs=w2l[:], start=True, stop=True,
                                             skip_group_check=True)
                        nc.scalar.copy(zt[:].rearrange("p b f -> p (b f)"), psz[:])
                        nc.sync.dma_start(
                            z2loc[w * 512:(w + 1) * 512, :].rearrange(
                                "(b p) f -> p b f", p=128), zt[:])
                    else:
                        nc.tensor.matmul(ps[:], lhsT=w2r[:], rhs=h1buf[:, sl],
                                         start=False, stop=True,
                                         skip_group_check=True)
                        h2 = php.tile([128, 512], FP16, tag="h2")
                        nc.scalar.activation(h2[:], ps[:], AF.Relu,
                                             bias=c2t[:, 0:1])
                        nc.sync.dma_start(hout_d[:, sl], h2[:])
                assert pc == NP

            for rep in range(nreps):
                # ---- phase A: z1 per edge-slot -> a2a_in ----
                for k in range(NKC):
                    xa = xsp.tile([128, 512], FP16, tag="xa")
                    xb = xsp.tile([2, 512], FP16, tag="xb")
                    nc.sync.dma_start(xa[:], xca_d[:, k * 512:(k + 1) * 512])
                    nc.sync.dma_start(xb[:], xcb_d[:, k * 512:(k + 1) * 512])
                    zt = ztp.tile([128, 4, 128], FP16, tag="zt")
                    ps = ps1p.tile([128, 512], F32, tag="ps1")
                    for b in range(4):
                        nc.tensor.matmul(ps[:, b * 128:(b + 1) * 128],
                                         lhsT=xa[:, b * 128:(b + 1) * 128],
                                         rhs=w1la[:], start=True, stop=False,
                                         skip_group_check=True)
                        nc.tensor.matmul(ps[:, b * 128:(b + 1) * 128],
                                         lhsT=xb[:, b * 128:(b + 1) * 128],
                                         rhs=w1lb[:], start=False, stop=True,
                                         skip_group_check=True)
                    nc.scalar.copy(zt[:].rearrange("p b f -> p (b f)"), ps[:])
                    nc.sync.dma_start(
                        a2a_in[k * 512:(k + 1) * 512, :].rearrange(
                            "(b p) f -> p b f", p=128), zt[:])

                nc.gpsimd.collective_compute(
                    "AllToAll", mybir.AluOpType.bypass,
                    replica_groups=[list(range(C))],
                    ins=[a2a_in[:].opt()], outs=[halo[:].opt()])

                aggregate(1)

                # ---- sender gather layer 2 ----
                for q in range(ACELL // 1024):
                    st = ztp.tile([128, 8, 128], FP16, tag="st")
                    nc.gpsimd.dma_gather(
                        out_ap=st[:], in_ap=z2loc[:],
                        idxs_ap=sidx[:, q * 64:(q + 1) * 64],
                        num_idxs=1024, num_idxs_reg=1024, elem_size=128,
                        queue_num=q % 4)
                    nc.sync.dma_start(
                        a2a_in[q * 1024:(q + 1) * 1024, :].rearrange(
                            "(t p) f -> p t f", p=128), st[:])

                nc.gpsimd.collective_compute(
                    "AllToAll", mybir.AluOpType.bypass,
                    replica_groups=[list(range(C))],
                    ins=[a2a_in[:].opt()], outs=[halo[:].opt()])

                aggregate(2)
    nc.compile()
    return nc


# ---------------------------------------------------------------------------
# PJRT runner (sharded over the 8 cores)
# ---------------------------------------------------------------------------

class _SpmdRunner:
    def __init__(self, nc, n_cores=C):
        import jax
        from jax.sharding import Mesh, PartitionSpec
        from jax.experimental.shard_map import shard_map
        import concourse.mybir as mybir
        from concourse.bass2jax import (_bass_exec_p, install_neuronx_cc_hook,
                                        partition_id_tensor)
        self.jax = jax
        install_neuronx_cc_hook()
        self.n_cores = n_cores
        partition_name = nc.partition_id_tensor.name if nc.partition_id_tensor else None
        in_names, out_names, out_avals, zero_outs = [], [], [], []
        for alloc in nc.m.functions[0].allocations:
            if not isinstance(alloc, mybir.MemoryLocationSet):
                continue
            name = alloc.memorylocations[0].name
            if alloc.kind == "ExternalInput":
                if name != partition_name:
                    in_names.append(name)
            elif alloc.kind == "ExternalOutput":
                shape = tuple(alloc.tensor_shape)
                dtype = mybir.dt.np(alloc.dtype)
                out_names.append(name)
                out_avals.append(jax.core.ShapedArray(shape, dtype))
                zero_outs.append(np.zeros(shape, dtype))
        self.in_names, self.out_names = in_names, out_names
        self.out_avals, self.zero_outs = out_avals, zero_outs
        n_params = len(in_names)
        all_in_names = list(in_names) + list(out_names)
        if partition_name is not None:
            all_in_names.append(partition_name)

        def _body(*args):
            operands = list(args)
            if partition_name is not None:
                operands.append(partition_id_tensor())
            outs = _bass_exec_p.bind(
                *operands,
                out_avals=tuple(out_avals),
                in_names=tuple(all_in_names),
                out_names=tuple(out_names),
                lowering_input_output_aliases=(),
                sim_require_finite=False,
                sim_require_nnan=False,
                nc=nc,
            )
            return tuple(outs)

        devices = jax.devices()[:n_cores]
        self.mesh = Mesh(np.asarray(devices), ("core",))
        in_specs = (PartitionSpec("core"),) * (n_params + len(out_names))
        out_specs = (PartitionSpec("core"),) * len(out_names)
        self.fn = jax.jit(
            shard_map(_body, mesh=self.mesh, in_specs=in_specs,
                      out_specs=out_specs, check_rep=False),
            keep_unused=True,
        )
        self.PartitionSpec = PartitionSpec

    def upload(self, in_maps):
        jax = self.jax
        n = self.n_cores
        per_core = [[np.asarray(m[name]) for name in self.in_names] for m in in_maps]
        concat_in = [np.concatenate([per_core[c][i] for c in range(n)], axis=0)
                     for i in range(len(self.in_names))]
        concat_zeros = [np.zeros((n * z.shape[0], *z.shape[1:]), z.dtype)
                        for z in self.zero_outs]
        sharding = jax.sharding.NamedSharding(self.mesh, self.PartitionSpec("core"))
        self.args = [jax.device_put(a, sharding) for a in concat_in + concat_zeros]

    def call_outputs(self):
        jax = self.jax
        outs = self.fn(*self.args)
        jax.block_until_ready(outs)
        n = self.n_cores
        return [
            {name: np.asarray(outs[i]).reshape(n, *self.out_avals[i].shape)[c]
             for i, name in enumerate(self.out_names)}
            for c in range(n)
        ]


def _prepare(edge_index, nreps=1):
    key = (hash(edge_index.tobytes()), nreps)
    if key in _cache:
        return _cache[key]
    g = _preprocess_graph(edge_index)
    nc = _build_kernel(g, nreps=nreps)
    runner = _SpmdRunner(nc)
    _cache[key] = (g, runner)
    return g, runner


def kernel(**inputs):
    edge_index = np.asarray(inputs['edge_index'])
    g, runner = _prepare(edge_index, nreps=1)
    in_maps = _make_in_maps(inputs, g)
    runner.upload(in_maps)
    outs = runner.call_outputs()
    return _assemble_output(outs, g)


# revision 5
# speedup vs baseline: 1.2670x; 1.0184x over previous
"""GraphSAGE 2-layer on 8 trn2 cores — v4: window-bucketed per-edge A2A halo.

Like v3 (per-edge AllToAll cells, one-hot scatter, no receiver gathers), but
every cell is subdivided into NWIN window-buckets of SHARED size BKT[w]
(max over all (core, shard) pairs, 128-aligned).  The receiver then works
window-major: for window w it streams the 8 cells' w-buckets (plain DMAs),
scatter-matmuls every 128-row tile into ONE accumulating PSUM window, and
flushes with a single tensor_copy into an fp16 agg.  Piece count == tile
count (no cross-core union), no flush adds, no agg memset.
Pad rows are masked by all-zero one-hot columns (dcmp == -1).
"""
import sys
sys.path.insert(0, '/opt/trn_rl_repo')

import numpy as np

N = 200000
E = 600000
C = 8
P = N // C
IN_DIM = 130
HID = 128
BN_EPS = 1e-5
WIN = 512
SL = 25088
NWIN = SL // WIN           # 49
F16 = np.float16

_cache = {}


def _round_up(x, m):
    return (x + m - 1) // m * m


def _wrap16(flat):
    s = len(flat)
    blk = np.asarray(flat, np.int16).reshape(s // 16, 16).T
    return np.tile(blk, (8, 1))


def _preprocess_graph(edge_index):
    src = edge_index[0].astype(np.int64)
    dst = edge_index[1].astype(np.int64)
    owner = dst // P
    dloc = dst - owner * P
    jsh = src // P
    wnd = dloc // WIN
    deg = np.bincount(dst, minlength=N)
    invd = (1.0 / np.maximum(deg, 1)).astype(np.float32)

    # bucket sizes shared across every (core, shard) pair
    cnt = np.zeros((C, C, NWIN), np.int64)
    np.add.at(cnt, (owner, jsh, wnd), 1)
    BKT = ((cnt.max(axis=(0, 1)) + 127) // 128) * 128      # [NWIN]
    OFF = np.concatenate(([0], np.cumsum(BKT)))            # [NWIN+1]
    CELLJ = int(OFF[-1])
    ACELL = C * CELLJ

    # order edges by (owner, shard, window, dloc); rank within run
    eorder = np.lexsort((dloc, wnd, jsh, owner))
    o_s, j_s, w_s, d_s, s_s = (owner[eorder], jsh[eorder], wnd[eorder],
                               dloc[eorder], src[eorder])
    key = (o_s * C + j_s) * NWIN + w_s
    bounds = np.searchsorted(key, np.arange(C * C * NWIN + 1))

    # per-halo-row tables (receiver view, core c)
    dcmp = np.full((C, ACELL), -1.0, np.float32)
    dinv = np.zeros((C, ACELL), np.float32)
    # sender view (core j): a2a_in row -> global src / src slot
    sendsrc = np.full((C, ACELL), -1, np.int64)
    sendflat = np.full((C, ACELL), SL, np.int64)
    for c in range(C):
        for j in range(C):
            for w in range(NWIN):
                k = (c * C + j) * NWIN + w
                a, b = bounds[k], bounds[k + 1]
                n = b - a
                base = j * CELLJ + OFF[w]
                dcmp[c, base:base + n] = d_s[a:b] - w * WIN
                dinv[c, base:base + n] = invd[c * P + d_s[a:b]]
                sbase = c * CELLJ + OFF[w]
                sendsrc[j, sbase:sbase + n] = s_s[a:b]
                sendflat[j, sbase:sbase + n] = s_s[a:b] - j * P
    sendidx = np.stack([_wrap16(sendflat[j]) for j in range(C)])

    # pieces: window-major, (w, j, t); NP = sum BKT/128 * C
    NP = int(BKT.sum() // 128) * C
    dcmp_in = np.empty((C, 128, NP), np.float32)
    dinv_in = np.empty((C, 128, NP), np.float32)
    pc = 0
    piece_plan = []            # (w, j, t, pc)
    for w in range(NWIN):
        for j in range(C):
            for t in range(int(BKT[w]) // 128):
                base = j * CELLJ + OFF[w] + t * 128
                dcmp_in[:, :, pc] = dcmp[:, base:base + 128]
                dinv_in[:, :, pc] = dinv[:, base:base + 128]
                piece_plan.append((w, j, t, pc))
                pc += 1
    assert pc == NP

    return dict(CELLJ=CELLJ, ACELL=ACELL, NP=NP, BKT=BKT.astype(int),
                OFF=OFF.astype(int), piece_plan=piece_plan,
                dcmp_in=dcmp_in, dinv_in=dinv_in,
                sendsrc=sendsrc, sendflat=sendflat, sendidx=sendidx)


def _fold_weights(Wl, bl, Wr, g, be, rm, rv):
    s = (np.asarray(g) / np.sqrt(np.asarray(rv) + BN_EPS)).astype(np.float32)
    Wl_f = (np.asarray(Wl) * s[None, :]).astype(np.float32)
    Wr_f = (np.asarray(Wr) * s[None, :]).astype(np.float32)
    c = ((np.asarray(bl) - np.asarray(rm)) * s + np.asarray(be)).astype(np.float32)
    return Wl_f, Wr_f, c


def _make_in_maps(inputs, g):
    x = np.asarray(inputs['x'], np.float32)
    W1l, W1r, c1 = _fold_weights(inputs['W1_l'], inputs['b1_l'], inputs['W1_r'],
                                 inputs['g1'], inputs['be1'], inputs['rm1'],
                                 inputs['rv1'])
    W2l, W2r, c2 = _fold_weights(inputs['W2_l'], inputs['b2_l'], inputs['W2_r'],
                                 inputs['g2'], inputs['be2'], inputs['rm2'],
                                 inputs['rv2'])
    ACELL = g['ACELL']
    shared = {
        'W1la': W1l[0:128].astype(F16), 'W1lb': W1l[128:130].astype(F16),
        'W1ra': W1r[0:128].astype(F16), 'W1rb': W1r[128:130].astype(F16),
        'W2l': W2l.astype(F16), 'W2r': W2r.astype(F16),
        'c1': c1.reshape(128, 1), 'c2': c2.reshape(128, 1),
    }
    in_maps = []
    for c in range(C):
        m = dict(shared)
        xn = np.zeros((IN_DIM, SL), np.float32)
        xn[:, :P] = x[c * P:(c + 1) * P].T
        m['xna'] = xn[0:128].astype(F16)
        m['xnb'] = xn[128:130].astype(F16)
        xc = np.zeros((IN_DIM, ACELL), np.float32)
        sel = g['sendsrc'][c] >= 0
        xc[:, sel] = x[g['sendsrc'][c][sel]].T
        m['xca'] = xc[0:128].astype(F16)
        m['xcb'] = xc[128:130].astype(F16)
        m['dcmp'] = g['dcmp_in'][c]
        m['dinv'] = g['dinv_in'][c]
        m['iota'] = np.tile(np.arange(WIN, dtype=np.float32), (128, 1))
        m['sendidx'] = g['sendidx'][c]
        in_maps.append(m)
    return in_maps


def _assemble_output(outs, g):
    h2 = np.empty((N, HID), np.float32)
    for c in range(C):
        h2[c * P:(c + 1) * P] = outs[c]['hout'][:, :P].T.astype(np.float32)
    return h2


# ---------------------------------------------------------------------------
# numpy emulation
# ---------------------------------------------------------------------------

def _emulate_full(in_maps, g):
    CELLJ = g['CELLJ']

    def f16(a):
        return a.astype(F16).astype(np.float32)

    W1la = in_maps[0]['W1la'].astype(np.float32)
    W1lb = in_maps[0]['W1lb'].astype(np.float32)
    W1ra = in_maps[0]['W1ra'].astype(np.float32)
    W1rb = in_maps[0]['W1rb'].astype(np.float32)
    W2l = in_maps[0]['W2l'].astype(np.float32)
    W2r = in_maps[0]['W2r'].astype(np.float32)
    c1 = in_maps[0]['c1'].ravel()
    c2 = in_maps[0]['c2'].ravel()

    def aggregate(zin_rows):
        halo = [np.concatenate([zin_rows[j][c * CELLJ:(c + 1) * CELLJ]
                                for j in range(C)]) for c in range(C)]
        aggs = []
        iota = np.arange(WIN, dtype=np.float32)
        for c in range(C):
            agg = np.zeros((128, SL), np.float32)
            dc = in_maps[c]['dcmp'].astype(np.float32)
            di = in_maps[c]['dinv'].astype(np.float32)
            for (w, j, t, pc) in g['piece_plan']:
                base = j * CELLJ + g['OFF'][w] + t * 128
                tile = halo[c][base:base + 128]
                oh = f16((iota[None, :] == dc[:, pc:pc + 1]) * di[:, pc:pc + 1])
                agg[:, w * WIN:(w + 1) * WIN] += tile.T @ oh
            aggs.append(f16(agg))
        return aggs

    zcells = []
    for c in range(C):
        xca = in_maps[c]['xca'].astype(np.float32)
        xcb = in_maps[c]['xcb'].astype(np.float32)
        zcells.append(f16(xca.T @ W1la + xcb.T @ W1lb))
    aggs = aggregate(zcells)
    h1s = []
    z2locs = []
    for c in range(C):
        xna = in_maps[c]['xna'].astype(np.float32)
        xnb = in_maps[c]['xnb'].astype(np.float32)
        r1 = W1ra.T @ xna + W1rb.T @ xnb
        h1 = f16(np.maximum(aggs[c] + r1 + c1[:, None], 0.0))
        h1s.append(h1)
        z2 = np.concatenate([f16(h1.T @ W2l), np.zeros((16, 128), np.float32)])
        z2locs.append(z2)
    zcells2 = [z2locs[c][g['sendflat'][c]] for c in range(C)]
    aggs2 = aggregate(zcells2)
    outs = []
    for c in range(C):
        r2 = W2r.T @ h1s[c]
        h2 = f16(np.maximum(aggs2[c] + r2 + c2[:, None], 0.0))
        outs.append({'hout': h2.astype(F16)})
    return outs


# ---------------------------------------------------------------------------
# bass kernel
# ---------------------------------------------------------------------------

def _build_kernel(g, nreps=1):
    import concourse.bacc as bacc
    import concourse.tile as tile
    import concourse.mybir as mybir

    F32 = mybir.dt.float32
    FP16 = mybir.dt.float16
    I16 = mybir.dt.int16
    AF = mybir.ActivationFunctionType
    ALU = mybir.AluOpType

    ACELL, CELLJ, NP = g['ACELL'], g['CELLJ'], g['NP']
    BKT, OFF = g['BKT'], g['OFF']
    NKC = ACELL // 512
    NW = SL // 512
    maxbt = int(max(BKT)) // 128

    nc = bacc.Bacc("TRN2", target_bir_lowering=False, debug=False, num_devices=C,
                   num_swdge_queues=4)

    xna_d = nc.dram_tensor("xna", [128, SL], FP16, kind="ExternalInput")
    xnb_d = nc.dram_tensor("xnb", [2, SL], FP16, kind="ExternalInput")
    xca_d = nc.dram_tensor("xca", [128, ACELL], FP16, kind="ExternalInput")
    xcb_d = nc.dram_tensor("xcb", [2, ACELL], FP16, kind="ExternalInput")
    W1la_d = nc.dram_tensor("W1la", [128, 128], FP16, kind="ExternalInput")
    W1lb_d = nc.dram_tensor("W1lb", [2, 128], FP16, kind="ExternalInput")
    W1ra_d = nc.dram_tensor("W1ra", [128, 128], FP16, kind="ExternalInput")
    W1rb_d = nc.dram_tensor("W1rb", [2, 128], FP16, kind="ExternalInput")
    W2l_d = nc.dram_tensor("W2l", [128, 128], FP16, kind="ExternalInput")
    W2r_d = nc.dram_tensor("W2r", [128, 128], FP16, kind="ExternalInput")
    c1_d = nc.dram_tensor("c1", [128, 1], F32, kind="ExternalInput")
    c2_d = nc.dram_tensor("c2", [128, 1], F32, kind="ExternalInput")
    dcmp_d = nc.dram_tensor("dcmp", [128, NP], F32, kind="ExternalInput")
    dinv_d = nc.dram_tensor("dinv", [128, NP], F32, kind="ExternalInput")
    iota_d = nc.dram_tensor("iota", [128, WIN], F32, kind="ExternalInput")
    sidx_d = nc.dram_tensor("sendidx", [128, ACELL // 16], I16, kind="ExternalInput")
    hout_d = nc.dram_tensor("hout", [128, SL], FP16, kind="ExternalOutput")

    a2a_in = nc.dram_tensor("a2ain", [ACELL, 128], FP16)
    halo = nc.dram_tensor("halo", [ACELL, 128], FP16)
    z2loc = nc.dram_tensor("z2loc", [SL + 16, 128], FP16)

    with tile.TileContext(nc) as tc:
        with (
            tc.tile_pool(name="const", bufs=1) as cons,
            tc.tile_pool(name="big", bufs=1) as bigp,
            tc.tile_pool(name="xs", bufs=4) as xsp,
            tc.tile_pool(name="zt", bufs=4) as ztp,
            tc.tile_pool(name="hc", bufs=12) as hcp,
            tc.tile_pool(name="oh", bufs=6) as ohp,
            tc.tile_pool(name="ph", bufs=3) as php,
            tc.tile_pool(name="agps", bufs=4, space="PSUM") as agps,
            tc.tile_pool(name="ps1", bufs=3, space="PSUM") as ps1p,
        ):
            w1la = cons.tile([128, 128], FP16)
            w1lb = cons.tile([2, 128], FP16)
            w1ra = cons.tile([128, 128], FP16)
            w1rb = cons.tile([2, 128], FP16)
            w2l = cons.tile([128, 128], FP16)
            w2r = cons.tile([128, 128], FP16)
            c1t = cons.tile([128, 1], F32)
            c2t = cons.tile([128, 1], F32)
            dcmp_t = cons.tile([128, NP], F32)
            dinv_t = cons.tile([128, NP], F32)
            iota_t = cons.tile([128, WIN], F32)
            sidx = cons.tile([128, ACELL // 16], I16)
            for sb, dr in ((w1la, W1la_d), (w1lb, W1lb_d), (w1ra, W1ra_d),
                           (w1rb, W1rb_d), (w2l, W2l_d), (w2r, W2r_d),
                           (c1t, c1_d), (c2t, c2_d), (dcmp_t, dcmp_d),
                           (dinv_t, dinv_d), (iota_t, iota_d), (sidx, sidx_d)):
                nc.sync.dma_start(sb[:], dr[:])
            zrow = cons.tile([1, 128], FP16)
            nc.vector.memset(zrow[:], 0.0)
            nc.sync.dma_start(z2loc[SL:SL + 1, :], zrow[:])

            h1buf = bigp.tile([128, SL], FP16)

            def aggregate(layer):
                pc = 0
                for w in range(NWIN):
                    sl = slice(w * 512, (w + 1) * 512)
                    ps = agps.tile([128, WIN], F32, tag="agps")
                    first = True
                    for j in range(C):
                        nt = int(BKT[w]) // 128
                        hch = hcp.tile([128, maxbt, 128], FP16, tag="hch")
                        base = j * CELLJ + int(OFF[w])
                        nc.sync.dma_start(
                            hch[:, 0:nt, :],
                            halo[base:base + nt * 128, :].rearrange(
                                "(b p) f -> p b f", p=128))
                        for t in range(nt):
                            oh = ohp.tile([128, WIN], FP16, tag="oh")
                            nc.vector.tensor_scalar(
                                oh[:], iota_t[:],
                                dcmp_t[:, pc:pc + 1], dinv_t[:, pc:pc + 1],
                                op0=ALU.is_equal, op1=ALU.mult)
                            nc.tensor.matmul(ps[:], lhsT=hch[:, t, :], rhs=oh[:],
                                             start=first, stop=False,
                                             skip_group_check=True)
                            first = False
                            pc += 1
                    if layer == 1:
                        xa = xsp.tile([128, 512], FP16, tag="xa")
                        xb = xsp.tile([2, 512], FP16, tag="xb")
                        nc.sync.dma_start(xa[:], xna_d[:, sl])
                        nc.sync.dma_start(xb[:], xnb_d[:, sl])
                        nc.tensor.matmul(ps[:], lhsT=w1ra[:], rhs=xa[:],
                                         start=False, stop=False,
                                         skip_group_check=True)
                        nc.tensor.matmul(ps[:], lhsT=w1rb[:], rhs=xb[:],
                                         start=False, stop=True,
                                         skip_group_check=True)
                        nc.scalar.activation(h1buf[:, sl], ps[:], AF.Relu,
                                             bias=c1t[:, 0:1])
                        zt = ztp.tile([128, 4, 128], FP16, tag="zt")
                        psz = ps1p.tile([128, 512], F32, tag="ps1")
                        for b in range(4):
                            nc.tensor.matmul(psz[:, b * 128:(b + 1) * 128],
                                             lhsT=h1buf[:, w * 512 + b * 128:
                                                        w * 512 + (b + 1) * 128],
                                             rhs=w2l[:], start=True, stop=True,
                                             skip_group_check=True)
                        nc.scalar.copy(zt[:].rearrange("p b f -> p (b f)"), psz[:])
                        nc.sync.dma_start(
                            z2loc[w * 512:(w + 1) * 512, :].rearrange(
                                "(b p) f -> p b f", p=128), zt[:])
                    else:
                        nc.tensor.matmul(ps[:], lhsT=w2r[:], rhs=h1buf[:, sl],
                                         start=False, stop=True,
                                         skip_group_check=True)
                        h2 = php.tile([128, 512], FP16, tag="h2")
                        nc.scalar.activation(h2[:], ps[:], AF.Relu,
                                             bias=c2t[:, 0:1])
                        nc.sync.dma_start(hout_d[:, sl], h2[:])
                assert pc == NP

            for rep in range(nreps):
                # ---- phase A: z1 per edge-slot -> a2a_in ----
                for k in range(NKC):
                    xa = xsp.tile([128, 512], FP16, tag="xa")
                    xb = xsp.tile([2, 512], FP16, tag="xb")
                    nc.sync.dma_start(xa[:], xca_d[:, k * 512:(k + 1) * 512])
                    nc.sync.dma_start(xb[:], xcb_d[:, k * 512:(k + 1) * 512])
                    zt = ztp.tile([128, 4, 128], FP16, tag="zt")
                    ps = ps1p.tile([128, 512], F32, tag="ps1")
                    for b in range(4):
                        nc.tensor.matmul(ps[:, b * 128:(b + 1) * 128],
                                         lhsT=xa[:, b * 128:(b + 1) * 128],
                                         rhs=w1la[:], start=True, stop=False,
                                         skip_group_check=True)
                        nc.tensor.matmul(ps[:, b * 128:(b + 1) * 128],
                                         lhsT=xb[:, b * 128:(b + 1) * 128],
                                         rhs=w1lb[:], start=False, stop=True,
                                         skip_group_check=True)
                    nc.scalar.copy(zt[:].rearrange("p b f -> p (b f)"), ps[:])
                    nc.sync.dma_start(
                        a2a_in[k * 512:(k + 1) * 512, :].rearrange(
                            "(b p) f -> p b f", p=128), zt[:])

                nc.gpsimd.collective_compute(
                    "AllToAll", mybir.AluOpType.bypass,
                    replica_groups=[list(range(C))],
                    ins=[a2a_in[:].opt()], outs=[halo[:].opt()])

                aggregate(1)

                # ---- sender gather layer 2 ----
                for q in range(ACELL // 1024):
                    st = ztp.tile([128, 8, 128], FP16, tag="st")
                    nc.gpsimd.dma_gather(
                        out_ap=st[:], in_ap=z2loc[:],
                        idxs_ap=sidx[:, q * 64:(q + 1) * 64],
                        num_idxs=1024, num_idxs_reg=1024, elem_size=128,
                        queue_num=q % 4)
                    nc.sync.dma_start(
                        a2a_in[q * 1024:(q + 1) * 1024, :].rearrange(
                            "(t p) f -> p t f", p=128), st[:])

                nc.gpsimd.collective_compute(
                    "AllToAll", mybir.AluOpType.bypass,
                    replica_groups=[list(range(C))],
                    ins=[a2a_in[:].opt()], outs=[halo[:].opt()])

                aggregate(2)
    nc.compile()
    return nc


# ---------------------------------------------------------------------------
# PJRT runner (sharded over the 8 cores)
# ---------------------------------------------------------------------------

class _SpmdRunner:
    def __init__(self, nc, n_cores=C):
        import jax
        from jax.sharding import Mesh, PartitionSpec
        from jax.experimental.shard_map import shard_map
        import concourse.mybir as mybir
        from concourse.bass2jax import (_bass_exec_p, install_neuronx_cc_hook,
                                        partition_id_tensor)
        self.jax = jax
        install_neuronx_cc_hook()
        self.n_cores = n_cores
        partition_name = nc.partition_id_tensor.name if nc.partition_id_tensor else None
        in_names, out_names, out_avals, zero_outs = [], [], [], []
        for alloc in nc.m.functions[0].allocations:
            if not isinstance(alloc, mybir.MemoryLocationSet):
                continue
            name = alloc.memorylocations[0].name
            if alloc.kind == "ExternalInput":
                if name != partition_name:
                    in_names.append(name)
            elif alloc.kind == "ExternalOutput":
                shape = tuple(alloc.tensor_shape)
                dtype = mybir.dt.np(alloc.dtype)
                out_names.append(name)
                out_avals.append(jax.core.ShapedArray(shape, dtype))
                zero_outs.append(np.zeros(shape, dtype))
        self.in_names, self.out_names = in_names, out_names
        self.out_avals, self.zero_outs = out_avals, zero_outs
        n_params = len(in_names)
        all_in_names = list(in_names) + list(out_names)
        if partition_name is not None:
            all_in_names.append(partition_name)

        def _body(*args):
            operands = list(args)
            if partition_name is not None:
                operands.append(partition_id_tensor())
            outs = _bass_exec_p.bind(
                *operands,
                out_avals=tuple(out_avals),
                in_names=tuple(all_in_names),
                out_names=tuple(out_names),
                lowering_input_output_aliases=(),
                sim_require_finite=False,
                sim_require_nnan=False,
                nc=nc,
            )
            return tuple(outs)

        devices = jax.devices()[:n_cores]
        self.mesh = Mesh(np.asarray(devices), ("core",))
        in_specs = (PartitionSpec("core"),) * (n_params + len(out_names))
        out_specs = (PartitionSpec("core"),) * len(out_names)
        self.fn = jax.jit(
            shard_map(_body, mesh=self.mesh, in_specs=in_specs,
                      out_specs=out_specs, check_rep=False),
            keep_unused=True,
        )
        self.PartitionSpec = PartitionSpec

    def upload(self, in_maps):
        jax = self.jax
        n = self.n_cores
        per_core = [[np.asarray(m[name]) for name in self.in_names] for m in in_maps]
        concat_in = [np.concatenate([per_core[c][i] for c in range(n)], axis=0)
                     for i in range(len(self.in_names))]
        concat_zeros = [np.zeros((n * z.shape[0], *z.shape[1:]), z.dtype)
                        for z in self.zero_outs]
        sharding = jax.sharding.NamedSharding(self.mesh, self.PartitionSpec("core"))
        self.args = [jax.device_put(a, sharding) for a in concat_in + concat_zeros]

    def call_outputs(self):
        jax = self.jax
        outs = self.fn(*self.args)
        jax.block_until_ready(outs)
        n = self.n_cores
        return [
            {name: np.asarray(outs[i]).reshape(n, *self.out_avals[i].shape)[c]
             for i, name in enumerate(self.out_names)}
            for c in range(n)
        ]


def _prepare(edge_index, nreps=1):
    key = (hash(edge_index.tobytes()), nreps)
    if key in _cache:
        return _cache[key]
    g = _preprocess_graph(edge_index)
    nc = _build_kernel(g, nreps=nreps)
    runner = _SpmdRunner(nc)
    _cache[key] = (g, runner)
    return g, runner


def kernel(**inputs):
    edge_index = np.asarray(inputs['edge_index'])
    g, runner = _prepare(edge_index, nreps=1)
    in_maps = _make_in_maps(inputs, g)
    runner.upload(in_maps)
    outs = runner.call_outputs()
    return _assemble_output(outs, g)
